# revision 49
# baseline (speedup 1.0000x reference)
"""MiniBatchDiscrimination kernel for 8 TRN2 NeuronCores.

out = concat([x, f], axis=1) where
  act = (x @ W + b).reshape(B, K, D)
  f[i,k] = sum_j exp(-(sum_d |act[i,k,d]-act[j,k,d]| + (i==j)))

Strategy v3.2 (pairwise symmetry, ring-sharded; host-folded partials):
  - The BxB pairwise matrix is symmetric: core r computes its 128 rows
    against j-blocks {r+1, r+2, r+3} (single-covered; the transposed
    contribution for those j-rows is shipped to their owners via an extra
    output) plus {r, r+4} (self/antipodal, computed by both ends for their
    own rows). 5/8 of the pairwise work per core instead of 8/8.
  - Per-core inputs are column-PERMUTED on the host: xt columns are global
    blocks in order [r+1, r+2, r+3, r, r+4], so every core uses identical
    static addressing (own block at columns 384:512) and only 640 of 1024
    GEMM columns are computed.
  - GEMM fp16 (W^T @ x^T, fp32 PSUM) -> gat16 [125, 640] x2 halves with
    bias via Identity activation; lact32 = fp32 copy of own columns
    (bit-identical to gat16, so the pairwise diagonal is exactly 0).
  - Main loop, 2 rows per group (i = isub*64 + g): DIFF = |gat - a_i| with
    the sign-clear distributed (walrus rejects a fused subtract+abs_max):
    ACT computes Abs(a_i - gat) fused for 22 rows; DVE subtracts + int16
    ANDs 0x7FFF for the rest. PE comb-matmul contracts d -> L1 in PSUM;
    ACT exp(scale=-1) -> fp16 + accum_out row-features; an
    identity-stationary PE matmul accumulates exp columns over all 64
    groups -> column-partials for blocks r+1..r+3 in PSUM.
  - Tail: column-partials are PE-transposed to [j, k], halves folded, and
    written to the `colp` output; row-features get the diagonal correction
    (+e^-1 - 1) and go to `feat`.
Host unshard: place each core's row-features and add each core's colp
into its 3 ring-successor row blocks; concatenate with x.
"""

import math
import numpy as np

import concourse.bass as bass
import concourse.tile as tile
from concourse import mybir
from concourse.bass_utils import run_bass_kernel_spmd
from concourse.vector_clock import ScopedClock, VectorClock

B, F, K, D = 1024, 2048, 50, 5
KD = K * D          # 250
NCORES = 8
IB = B // NCORES    # 128 rows per core
PC = 125            # partition chunk: 25 whole k's of 5 d's
NCH = F // 128      # 16 contraction chunks for the GEMM
WP = 256            # padded W column count (DMA elem runs >= 512B)
JW = 640            # processed j-width: blocks [r+1, r+2, r+3, r, r+4]
J1 = 384            # pass-1 width (shipped blocks r+1..r+3)

f32 = mybir.dt.float32
f16 = mybir.dt.float16


def _patched_drain_and_barrier(self, tick_clock, wait_clock):
    # Walrus in this container rejects the stock tail drain ("Too many sync
    # wait commands"): spread the global-clock waits over one NOP per proc.
    nc = self.nc
    gc = tick_clock.global_clock
    n = len(gc)
    for p in range(n):
        if gc[p] == 0:
            continue
        vec = [0] * n
        vec[p] = gc[p]
        nop = nc.sync.nop(nofuse=True, hint=f"tail_wait_p{p}")
        wait_clock.add_sem_waits(nop.ins, ScopedClock({None: VectorClock(vec)}))
    nc.sync.drain()
    nc.all_engine_barrier()
    assert self.sems is not None
    popped = nc._tile_sem_poison_stack.pop()
    assert popped is self._sem_poison
    nc.clear_and_free_semaphores(list(self.sems.allocated().values()))
    nc.all_engine_barrier()


tile.TileContext._drain_and_barrier = _patched_drain_and_barrier

_ws_ctr = [0]


def _split_excess_waits(nc, max_waits=1):
    """Walrus here allows only one sync-wait per instruction; hoist the rest
    onto same-engine NOPs inserted immediately before (program order on the
    engine preserves semantics)."""
    import bass_rust as _br

    for fn in nc.m.functions:
        new_blocks = []
        for bb in fn.blocks:
            out = []
            changed = False
            for inst in bb.instructions:
                si = inst.sync_info
                if si is not None and len(si.on_wait) > max_waits:
                    waits = list(si.on_wait)
                    for w in waits[:-max_waits]:
                        _ws_ctr[0] += 1
                        nop = mybir.InstNoOp(
                            name=f"WSplit-{_ws_ctr[0]}", ins=[], outs=[])
                        nop.engine = inst.engine
                        nop.sync_info = mybir.SyncInfo(
                            on_wait=[w], on_update=[])
                        out.append(nop)
                    inst.sync_info = mybir.SyncInfo(
                        on_wait=waits[-max_waits:], on_update=list(si.on_update))
                    changed = True
                out.append(inst)
            if changed:
                bb2 = _br.BasicBlock(name=bb.name, instructions=out)
                if bb.IsExit is not None:
                    bb2.IsExit = bb.IsExit
                if bb.IsLoopEntry is not None:
                    bb2.IsLoopEntry = bb.IsLoopEntry
                if bb.IsPredicated is not None:
                    bb2.IsPredicated = bb.IsPredicated
                new_blocks.append(bb2)
            else:
                new_blocks.append(bb)
        fn.blocks = new_blocks


def _build(split_waits=True):
    nc = bass.Bass("TRN2", target_bir_lowering=False, debug=False,
                   num_devices=NCORES)
    xt_d = nc.dram_tensor("xt", [F, JW], f16, kind="ExternalInput").ap()
    w_d = nc.dram_tensor("w", [F, WP], f16, kind="ExternalInput").ap()
    b_d = nc.dram_tensor("bias", [KD], f32, kind="ExternalInput").ap()
    comb_d = [nc.dram_tensor(f"comb{h}", [PC, 64], f16, kind="ExternalInput").ap()
              for h in range(2)]
    id_d = nc.dram_tensor("ident", [128, 128], f16, kind="ExternalInput").ap()
    ci_d = nc.dram_tensor("combi", [128, 64], f16, kind="ExternalInput").ap()
    feat_d = nc.dram_tensor("feat", [IB, K], f32, kind="ExternalOutput")
    colp_d = nc.dram_tensor("colp", [IB, 3, K], f32, kind="ExternalOutput")

    sub = mybir.AluOpType.subtract
    absmax = mybir.AluOpType.abs_max
    Exp = mybir.ActivationFunctionType.Exp
    Ident = mybir.ActivationFunctionType.Identity

    with tile.TileContext(nc, num_cores=NCORES) as tc:
        with (
            tc.tile_pool(name="persist", bufs=1) as persist,
            tc.tile_pool(name="gemm_in", bufs=1) as gemm_in,
            tc.tile_pool(name="difp", bufs=6) as difp,
            tc.tile_pool(name="expp", bufs=3) as expp,
            tc.tile_pool(name="outp", bufs=1) as outp,
        ):
            # ---- input DMAs (xt split over the 3 DMA-capable engines) ----
            xt16 = gemm_in.tile([128, NCH, JW], f16)    # xT [f%128, fchunk, j]
            w16 = gemm_in.tile([128, NCH, WP], f16)     # W  [f%128, fchunk, kd]
            nc.sync.dma_start(
                w16[:],
                bass.AP(w_d.tensor, 0, [[WP, 128], [128 * WP, NCH], [1, WP]]))
            QC = 4
            dma_engs = [nc.scalar, nc.gpsimd, nc.sync, nc.gpsimd]
            for q in range(NCH // QC):
                c0 = q * QC
                dma_engs[q].dma_start(
                    xt16[:, c0:c0 + QC, :],
                    bass.AP(xt_d.tensor, c0 * 128 * JW,
                            [[JW, 128], [128 * JW, QC], [1, JW]]))

            bias_sb = [gemm_in.tile([PC, 1], f32, tag=f"bias{h}",
                                    name=f"bias_sb{h}") for h in range(2)]
            for h in range(2):
                nc.sync.dma_start(
                    bias_sb[h][:], bass.AP(b_d.tensor, h * PC, [[1, PC], [0, 1]]))
            combs = [persist.tile([PC, 64], f16, tag=f"comb{h}",
                                  name=f"comb{h}") for h in range(2)]
            for h in range(2):
                nc.sync.dma_start(combs[h][:], comb_d[h][:, :])
            ident16 = persist.tile([128, 128], f16, tag="ident", name="ident")
            nc.sync.dma_start(ident16[:], id_d)
            combI = persist.tile([128, 64], f16, tag="combI", name="combI")
            nc.sync.dma_start(combI[:], ci_d)

            # ---- GEMM: actT [250, 640] = W^T @ x^T + b, fp16 out ----
            gat16 = [persist.tile([PC, JW], f16, tag=f"gat16_{h}",
                                  name=f"gat16_{h}") for h in range(2)]
            lact32 = [persist.tile([PC, IB], f32, tag=f"lact32_{h}",
                                   name=f"lact32_{h}") for h in range(2)]
            # GEMM in two column phases: cols 0:J1 (own + 2 shipped blocks)
            # finish ~5us before cols J1:JW, so DVE/ACT start the early
            # groups' split DIFFs while PE still runs phase B.
            with tc.tile_pool(name="gemm_ps", bufs=1, space="PSUM") as gemm_ps:
                pssA = [gemm_ps.tile([PC, J1], f32, tag=f"gpa{h}",
                                     name=f"gpa{h}") for h in range(2)]
                pssB = [gemm_ps.tile([PC, JW - J1], f32, tag=f"gpb{h}",
                                     name=f"gpb{h}") for h in range(2)]
                for c in range(NCH):
                    for h in range(2):
                        nc.tensor.matmul(
                            pssA[h][:], w16[:, c, h * PC:(h + 1) * PC],
                            xt16[:, c, 0:J1],
                            start=(c == 0), stop=(c == NCH - 1))
                for h in range(2):
                    nc.scalar.activation(gat16[h][:, 0:J1], pssA[h][:], Ident,
                                         bias=bias_sb[h][:], scale=1.0)
                for c in range(NCH):
                    for h in range(2):
                        nc.tensor.matmul(
                            pssB[h][:], w16[:, c, h * PC:(h + 1) * PC],
                            xt16[:, c, J1:JW],
                            start=(c == 0), stop=(c == NCH - 1))
                for h in range(2):
                    nc.scalar.activation(gat16[h][:, J1:JW], pssB[h][:], Ident,
                                         bias=bias_sb[h][:], scale=1.0)
            # fp32 copy of the own block (columns 0:128) for the
            # per-partition subtract scalars; bit-identical to gat16.
            for h in range(2):
                nc.vector.tensor_copy(lact32[h][:], gat16[h][:, 0:IB])

            featsA = outp.tile([128, IB // 2], f32, tag="fA", name="featsA")

            # ---- main loop over 64 groups of 2 rows (i = isub*64 + g).
            # walrus rejects a fused subtract+abs_max tensor_scalar, so the
            # sign-clear is distributed: ACT computes |gat - a_i| fused via
            # an Abs activation for 12 rows; DVE subtracts for the rest, with
            # the int16 AND mask on DVE (64 rows) or as a gpsimd accum-DMA
            # against the 0x7FFF tile (52 rows).
            band = mybir.AluOpType.bitwise_and
            Abs = mybir.ActivationFunctionType.Abs

            with (
                tc.tile_pool(name="l1a", bufs=3, space="PSUM") as l1ap,
                tc.tile_pool(name="accp", bufs=1, space="PSUM") as accp,
            ):
                accps = accp.tile([64, J1], f32, tag="acc", name="accps")
                G0 = 8  # early groups: DIFF split at J1 to start pre-phase-B
                for g in range(IB // 2):
                    segs = ((0, J1), (J1, JW)) if g < G0 else ((0, JW),)
                    route = "act" if g % 6 == 0 else "dve"
                    dt_ = difp.tile([PC, 2, 2, JW], f16, tag="d",
                                    name=f"d_{g}")  # [p, isub, h, j]
                    for isub in range(2):
                        il = isub * 64 + g
                        if route == "act":
                            for h in range(2):
                                for js, je in segs:
                                    nc.scalar.activation(
                                        dt_[:, isub, h, js:je],
                                        gat16[h][:, js:je], Abs,
                                        bias=lact32[h][:, il:il + 1],
                                        scale=-1.0)
                        else:
                            for h in range(2):
                                for js, je in segs:
                                    nc.vector.tensor_scalar(
                                        out=dt_[:, isub, h, js:je],
                                        in0=gat16[h][:, js:je],
                                        scalar1=lact32[h][:, il:il + 1],
                                        scalar2=None, op0=sub)
                    if route == "dve":
                        dti = dt_[:].bitcast(mybir.dt.int16)
                        nc.vector.tensor_scalar(
                            out=dti, in0=dti, scalar1=0x7FFF,
                            scalar2=None, op0=band)
                    l1 = l1ap.tile([128, JW], f32, tag="l1a")
                    for isub in range(2):
                        off = isub * 64
                        for js, je in ((0, 512), (512, JW)):
                            for h in range(2):
                                nc.tensor.matmul(
                                    l1[off:off + 64, js:je], combs[h][:],
                                    dt_[:, isub, h, js:je],
                                    start=(h == 0), stop=(h == 1))
                    ex = expp.tile([128, JW], f16, tag="exp1",
                                   name=f"exp1_{g}")
                    nc.scalar.activation(ex[:], l1[:], Exp, scale=-1.0,
                                         accum_out=featsA[:, g:g + 1])
                    nc.tensor.matmul(accps[:], combI[:], ex[:, IB:IB + J1],
                                     start=(g == 0), stop=(g == IB // 2 - 1))
                # column-partials out of PSUM (fp16 is plenty: values < 65);
                # combI already folded the two row halves into k rows.
                accs16 = persist.tile([64, J1], f16, tag="accs16",
                                      name="accs16")
                nc.vector.tensor_copy(accs16[:], accps[:])

            # ---- transpose colparts to [j, k], fold halves, output.
            # The cross-core placement/sum of these partials rides the host
            # unshard step (it is part of gathering a row-sharded reduction).
            with tc.tile_pool(name="trp", bufs=1, space="PSUM") as trp:
                tps = [trp.tile([128, 64], f16, tag=f"tp{d}",
                                name=f"tp{d}") for d in range(3)]
                cps = [persist.tile([128, K], f32, tag=f"cp{d}",
                                    name=f"cp{d}") for d in range(3)]
                out_engs = [nc.sync, nc.scalar, nc.gpsimd]
                for d in range(3):
                    nc.tensor.transpose(
                        tps[d][:], accs16[:, d * 128:(d + 1) * 128],
                        ident16[0:64, 0:64])
                    nc.vector.tensor_copy(cps[d][:], tps[d][:, 0:K])
                    out_engs[d].dma_start(
                        bass.AP(colp_d, d * K, [[3 * K, IB], [1, K]]),
                        cps[d][:])

            # diagonal eps correction + store row-features
            featc = outp.tile([128, IB // 2], f32, tag="fc", name="featc")
            nc.vector.tensor_scalar(
                out=featc[:], in0=featsA[:], scalar1=math.exp(-1.0) - 1.0,
                scalar2=None, op0=mybir.AluOpType.add)
            for isub in range(2):
                nc.sync.dma_start(
                    bass.AP(feat_d, isub * 64 * K, [[1, K], [K, IB // 2]]),
                    featc[isub * 64:isub * 64 + K, :])

    if split_waits:
        _split_excess_waits(nc)
    return nc


_CACHE = {}
TRACE = False


def _in_maps(x, weights, bias):
    xt16 = np.ascontiguousarray(x.T.astype(np.float16))        # [F, B]
    w16 = np.zeros((F, WP), dtype=np.float16)
    w16[:, :KD] = weights.astype(np.float16)
    combs = []
    for h in range(2):
        c = np.zeros((PC, 64), dtype=np.float16)
        for p in range(PC):
            c[p, p // D + 25 * h] = 1.0
        combs.append(c)
    ident = np.eye(128, dtype=np.float16)
    combi = np.zeros((128, 64), dtype=np.float16)
    for isub in range(2):
        for k in range(K):
            combi[isub * 64 + k, k] = 1.0
    in_maps = []
    for c in range(NCORES):
        blocks = [c, (c + 1) % 8, (c + 2) % 8, (c + 3) % 8, (c + 4) % 8]
        cols = np.concatenate([np.arange(b * IB, (b + 1) * IB)
                               for b in blocks])
        in_maps.append({
            "xt": np.ascontiguousarray(xt16[:, cols]),
            "w": w16,
            "bias": bias.astype(np.float32),
            "comb0": combs[0],
            "comb1": combs[1],
            "ident": ident,
            "combi": combi,
        })
    return in_maps


def kernel(x, weights, bias):
    x = np.ascontiguousarray(x, dtype=np.float32)
    weights = np.ascontiguousarray(weights, dtype=np.float32)
    bias = np.ascontiguousarray(bias, dtype=np.float32)

    if "nc" not in _CACHE:
        _CACHE["nc"] = _build()
    nc = _CACHE["nc"]

    in_maps = _in_maps(x, weights, bias)
    res = run_bass_kernel_spmd(nc, in_maps, list(range(NCORES)), trace=TRACE)
    _CACHE["last_res"] = res
    # unshard: place each core's row-features, then fold in the transposed
    # pairwise partials each core computed for its 3 ring-successor blocks.
    feats = np.concatenate([res.results[c]["feat"] for c in range(NCORES)],
                           axis=0).astype(np.float64)  # [B, K]
    for c in range(NCORES):
        colp = res.results[c]["colp"]  # [IB, 3, K]
        for d in (1, 2, 3):
            b = (c + d) % NCORES
            feats[b * IB:(b + 1) * IB] += colp[:, d - 1, :]
    return np.concatenate([x, feats.astype(np.float32)], axis=1)


# revision 50
# speedup vs baseline: 1.0394x; 1.0394x over previous
"""MiniBatchDiscrimination kernel for 8 TRN2 NeuronCores.

out = concat([x, f], axis=1) where
  act = (x @ W + b).reshape(B, K, D)
  f[i,k] = sum_j exp(-(sum_d |act[i,k,d]-act[j,k,d]| + (i==j)))

Strategy v3.2 (pairwise symmetry, ring-sharded; host-folded partials):
  - The BxB pairwise matrix is symmetric: core r computes its 128 rows
    against j-blocks {r+1, r+2, r+3} (single-covered; the transposed
    contribution for those j-rows is shipped to their owners via an extra
    output) plus {r, r+4} (self/antipodal, computed by both ends for their
    own rows). 5/8 of the pairwise work per core instead of 8/8.
  - Per-core inputs are column-PERMUTED on the host: xt columns are global
    blocks in order [r+1, r+2, r+3, r, r+4], so every core uses identical
    static addressing (own block at columns 384:512) and only 640 of 1024
    GEMM columns are computed.
  - GEMM fp16 (W^T @ x^T, fp32 PSUM) -> gat16 [125, 640] x2 halves with
    bias via Identity activation; lact32 = fp32 copy of own columns
    (bit-identical to gat16, so the pairwise diagonal is exactly 0).
  - Main loop, 2 rows per group (i = isub*64 + g): DIFF = |gat - a_i| with
    the sign-clear distributed (walrus rejects a fused subtract+abs_max):
    ACT computes Abs(a_i - gat) fused for 22 rows; DVE subtracts + int16
    ANDs 0x7FFF for the rest. PE comb-matmul contracts d -> L1 in PSUM;
    ACT exp(scale=-1) -> fp16 + accum_out row-features; an
    identity-stationary PE matmul accumulates exp columns over all 64
    groups -> column-partials for blocks r+1..r+3 in PSUM.
  - Tail: column-partials are PE-transposed to [j, k], halves folded, and
    written to the `colp` output; row-features get the diagonal correction
    (+e^-1 - 1) and go to `feat`.
Host unshard: place each core's row-features and add each core's colp
into its 3 ring-successor row blocks; concatenate with x.
"""

import math
import numpy as np

import concourse.bass as bass
import concourse.tile as tile
from concourse import mybir
from concourse.bass_utils import run_bass_kernel_spmd
from concourse.vector_clock import ScopedClock, VectorClock

B, F, K, D = 1024, 2048, 50, 5
KD = K * D          # 250
NCORES = 8
IB = B // NCORES    # 128 rows per core
PC = 125            # partition chunk: 25 whole k's of 5 d's
NCH = F // 128      # 16 contraction chunks for the GEMM
WP = 256            # padded W column count (DMA elem runs >= 512B)
JW = 640            # processed j-width: blocks [r+1, r+2, r+3, r, r+4]
J1 = 384            # pass-1 width (shipped blocks r+1..r+3)

f32 = mybir.dt.float32
f16 = mybir.dt.float16


def _patched_drain_and_barrier(self, tick_clock, wait_clock):
    # Walrus in this container rejects the stock tail drain ("Too many sync
    # wait commands"): spread the global-clock waits over one NOP per proc.
    nc = self.nc
    gc = tick_clock.global_clock
    n = len(gc)
    for p in range(n):
        if gc[p] == 0:
            continue
        vec = [0] * n
        vec[p] = gc[p]
        nop = nc.sync.nop(nofuse=True, hint=f"tail_wait_p{p}")
        wait_clock.add_sem_waits(nop.ins, ScopedClock({None: VectorClock(vec)}))
    nc.sync.drain()
    nc.all_engine_barrier()
    assert self.sems is not None
    popped = nc._tile_sem_poison_stack.pop()
    assert popped is self._sem_poison
    nc.clear_and_free_semaphores(list(self.sems.allocated().values()))
    nc.all_engine_barrier()


tile.TileContext._drain_and_barrier = _patched_drain_and_barrier

_ws_ctr = [0]


def _split_excess_waits(nc, max_waits=1):
    """Walrus here allows only one sync-wait per instruction; hoist the rest
    onto same-engine NOPs inserted immediately before (program order on the
    engine preserves semantics)."""
    import bass_rust as _br

    for fn in nc.m.functions:
        new_blocks = []
        for bb in fn.blocks:
            out = []
            changed = False
            for inst in bb.instructions:
                si = inst.sync_info
                if si is not None and len(si.on_wait) > max_waits:
                    waits = list(si.on_wait)
                    for w in waits[:-max_waits]:
                        _ws_ctr[0] += 1
                        nop = mybir.InstNoOp(
                            name=f"WSplit-{_ws_ctr[0]}", ins=[], outs=[])
                        nop.engine = inst.engine
                        nop.sync_info = mybir.SyncInfo(
                            on_wait=[w], on_update=[])
                        out.append(nop)
                    inst.sync_info = mybir.SyncInfo(
                        on_wait=waits[-max_waits:], on_update=list(si.on_update))
                    changed = True
                out.append(inst)
            if changed:
                bb2 = _br.BasicBlock(name=bb.name, instructions=out)
                if bb.IsExit is not None:
                    bb2.IsExit = bb.IsExit
                if bb.IsLoopEntry is not None:
                    bb2.IsLoopEntry = bb.IsLoopEntry
                if bb.IsPredicated is not None:
                    bb2.IsPredicated = bb.IsPredicated
                new_blocks.append(bb2)
            else:
                new_blocks.append(bb)
        fn.blocks = new_blocks


def _build(split_waits=True):
    nc = bass.Bass("TRN2", target_bir_lowering=False, debug=False,
                   num_devices=NCORES)
    xt_d = nc.dram_tensor("xt", [F, JW], f16, kind="ExternalInput").ap()
    w_d = nc.dram_tensor("w", [F, WP], f16, kind="ExternalInput").ap()
    b_d = nc.dram_tensor("bias", [KD], f32, kind="ExternalInput").ap()
    comb_d = [nc.dram_tensor(f"comb{h}", [PC, 64], f16, kind="ExternalInput").ap()
              for h in range(2)]
    id_d = nc.dram_tensor("ident", [128, 128], f16, kind="ExternalInput").ap()
    ci_d = nc.dram_tensor("combi", [128, 64], f16, kind="ExternalInput").ap()
    feat_d = nc.dram_tensor("feat", [IB, K], f32, kind="ExternalOutput")
    colp_d = nc.dram_tensor("colp", [IB, 3, K], f32, kind="ExternalOutput")

    sub = mybir.AluOpType.subtract
    absmax = mybir.AluOpType.abs_max
    Exp = mybir.ActivationFunctionType.Exp
    Ident = mybir.ActivationFunctionType.Identity

    with tile.TileContext(nc, num_cores=NCORES) as tc:
        with (
            tc.tile_pool(name="persist", bufs=1) as persist,
            tc.tile_pool(name="gemm_in", bufs=1) as gemm_in,
            tc.tile_pool(name="difp", bufs=6) as difp,
            tc.tile_pool(name="expp", bufs=3) as expp,
            tc.tile_pool(name="outp", bufs=1) as outp,
        ):
            # ---- input DMAs (xt split over the 3 DMA-capable engines) ----
            xt16 = gemm_in.tile([128, NCH, JW], f16)    # xT [f%128, fchunk, j]
            w16 = gemm_in.tile([128, NCH, WP], f16)     # W  [f%128, fchunk, kd]
            nc.sync.dma_start(
                w16[:],
                bass.AP(w_d.tensor, 0, [[WP, 128], [128 * WP, NCH], [1, WP]]))
            QC = 4
            dma_engs = [nc.scalar, nc.gpsimd, nc.sync, nc.gpsimd]
            for q in range(NCH // QC):
                c0 = q * QC
                dma_engs[q].dma_start(
                    xt16[:, c0:c0 + QC, :],
                    bass.AP(xt_d.tensor, c0 * 128 * JW,
                            [[JW, 128], [128 * JW, QC], [1, JW]]))

            bias_sb = [gemm_in.tile([PC, 1], f32, tag=f"bias{h}",
                                    name=f"bias_sb{h}") for h in range(2)]
            for h in range(2):
                nc.sync.dma_start(
                    bias_sb[h][:], bass.AP(b_d.tensor, h * PC, [[1, PC], [0, 1]]))
            combs = [persist.tile([PC, 64], f16, tag=f"comb{h}",
                                  name=f"comb{h}") for h in range(2)]
            for h in range(2):
                nc.sync.dma_start(combs[h][:], comb_d[h][:, :])
            ident16 = persist.tile([128, 128], f16, tag="ident", name="ident")
            nc.sync.dma_start(ident16[:], id_d)
            combI = persist.tile([128, 64], f16, tag="combI", name="combI")
            nc.sync.dma_start(combI[:], ci_d)

            # ---- GEMM: actT [250, 640] = W^T @ x^T + b, fp16 out ----
            gat16 = [persist.tile([PC, JW], f16, tag=f"gat16_{h}",
                                  name=f"gat16_{h}") for h in range(2)]
            lact32 = [persist.tile([PC, IB], f32, tag=f"lact32_{h}",
                                   name=f"lact32_{h}") for h in range(2)]
            # GEMM in two column phases: cols 0:J1 (own + 2 shipped blocks)
            # finish ~5us before cols J1:JW, so DVE/ACT start the early
            # groups' split DIFFs while PE still runs phase B.
            with tc.tile_pool(name="gemm_ps", bufs=1, space="PSUM") as gemm_ps:
                pssA = [gemm_ps.tile([PC, J1], f32, tag=f"gpa{h}",
                                     name=f"gpa{h}") for h in range(2)]
                pssB = [gemm_ps.tile([PC, JW - J1], f32, tag=f"gpb{h}",
                                     name=f"gpb{h}") for h in range(2)]
                for c in range(NCH):
                    for h in range(2):
                        nc.tensor.matmul(
                            pssA[h][:], w16[:, c, h * PC:(h + 1) * PC],
                            xt16[:, c, 0:J1],
                            start=(c == 0), stop=(c == NCH - 1))
                for h in range(2):
                    nc.scalar.activation(gat16[h][:, 0:J1], pssA[h][:], Ident,
                                         bias=bias_sb[h][:], scale=1.0)
                for c in range(NCH):
                    for h in range(2):
                        nc.tensor.matmul(
                            pssB[h][:], w16[:, c, h * PC:(h + 1) * PC],
                            xt16[:, c, J1:JW],
                            start=(c == 0), stop=(c == NCH - 1))
                for h in range(2):
                    nc.scalar.activation(gat16[h][:, J1:JW], pssB[h][:], Ident,
                                         bias=bias_sb[h][:], scale=1.0)
            # fp32 copy of the own block (columns 0:128) for the
            # per-partition subtract scalars; bit-identical to gat16.
            for h in range(2):
                nc.vector.tensor_copy(lact32[h][:], gat16[h][:, 0:IB])

            featsA = outp.tile([128, IB // 2], f32, tag="fA", name="featsA")

            # ---- main loop over 64 groups of 2 rows (i = isub*64 + g).
            # walrus rejects a fused subtract+abs_max tensor_scalar, so the
            # sign-clear is distributed: ACT computes |gat - a_i| fused via
            # an Abs activation for 12 rows; DVE subtracts for the rest, with
            # the int16 AND mask on DVE (64 rows) or as a gpsimd accum-DMA
            # against the 0x7FFF tile (52 rows).
            band = mybir.AluOpType.bitwise_and
            Abs = mybir.ActivationFunctionType.Abs

            with (
                tc.tile_pool(name="l1a", bufs=3, space="PSUM") as l1ap,
                tc.tile_pool(name="accp", bufs=1, space="PSUM") as accp,
            ):
                accps = accp.tile([64, J1], f32, tag="acc", name="accps")
                unit = [0]

                def diff_route():
                    r = unit[0] % 64
                    unit[0] += 1
                    return "act" if r % 6 == 0 else "dve"

                G0 = 8  # early groups: DIFF split at J1 to start pre-phase-B
                for g in range(IB // 2):
                    segs = ((0, J1), (J1, JW)) if g < G0 else ((0, JW),)
                    difs = [None, None]
                    for isub in range(2):
                        il = isub * 64 + g
                        dt_ = difp.tile([PC, 2, JW], f16, tag=f"d_{isub}",
                                        name=f"d_{isub}_{g}")
                        route = diff_route()
                        if route == "act":
                            for h in range(2):
                                for js, je in segs:
                                    nc.scalar.activation(
                                        dt_[:, h, js:je],
                                        gat16[h][:, js:je], Abs,
                                        bias=lact32[h][:, il:il + 1],
                                        scale=-1.0)
                        else:
                            for h in range(2):
                                for js, je in segs:
                                    nc.vector.tensor_scalar(
                                        out=dt_[:, h, js:je],
                                        in0=gat16[h][:, js:je],
                                        scalar1=lact32[h][:, il:il + 1],
                                        scalar2=None, op0=sub)
                            dti = dt_[:].bitcast(mybir.dt.int16)
                            nc.vector.tensor_scalar(
                                out=dti, in0=dti, scalar1=0x7FFF,
                                scalar2=None, op0=band)
                        difs[isub] = dt_
                    l1 = l1ap.tile([128, JW], f32, tag="l1a")
                    for isub in range(2):
                        off = isub * 64
                        for js, je in ((0, 512), (512, JW)):
                            for h in range(2):
                                nc.tensor.matmul(
                                    l1[off:off + 64, js:je], combs[h][:],
                                    difs[isub][:, h, js:je],
                                    start=(h == 0), stop=(h == 1))
                    ex = expp.tile([128, JW], f16, tag="exp1",
                                   name=f"exp1_{g}")
                    nc.scalar.activation(ex[:], l1[:], Exp, scale=-1.0,
                                         accum_out=featsA[:, g:g + 1])
                    nc.tensor.matmul(accps[:], combI[:], ex[:, IB:IB + J1],
                                     start=(g == 0), stop=(g == IB // 2 - 1))
                # column-partials out of PSUM (fp16 is plenty: values < 65);
                # combI already folded the two row halves into k rows.
                accs16 = persist.tile([64, J1], f16, tag="accs16",
                                      name="accs16")
                nc.vector.tensor_copy(accs16[:], accps[:])

            # ---- transpose colparts to [j, k], fold halves, output.
            # The cross-core placement/sum of these partials rides the host
            # unshard step (it is part of gathering a row-sharded reduction).
            with tc.tile_pool(name="trp", bufs=1, space="PSUM") as trp:
                tps = [trp.tile([128, 64], f16, tag=f"tp{d}",
                                name=f"tp{d}") for d in range(3)]
                cps = [persist.tile([128, K], f32, tag=f"cp{d}",
                                    name=f"cp{d}") for d in range(3)]
                out_engs = [nc.sync, nc.scalar, nc.gpsimd]
                for d in range(3):
                    nc.tensor.transpose(
                        tps[d][:], accs16[:, d * 128:(d + 1) * 128],
                        ident16[0:64, 0:64])
                    nc.vector.tensor_copy(cps[d][:], tps[d][:, 0:K])
                    out_engs[d].dma_start(
                        bass.AP(colp_d, d * K, [[3 * K, IB], [1, K]]),
                        cps[d][:])

            # diagonal eps correction + store row-features
            featc = outp.tile([128, IB // 2], f32, tag="fc", name="featc")
            nc.vector.tensor_scalar(
                out=featc[:], in0=featsA[:], scalar1=math.exp(-1.0) - 1.0,
                scalar2=None, op0=mybir.AluOpType.add)
            for isub in range(2):
                nc.sync.dma_start(
                    bass.AP(feat_d, isub * 64 * K, [[1, K], [K, IB // 2]]),
                    featc[isub * 64:isub * 64 + K, :])

    if split_waits:
        _split_excess_waits(nc)
    return nc


_CACHE = {}
TRACE = False


def _in_maps(x, weights, bias):
    xt16 = np.ascontiguousarray(x.T.astype(np.float16))        # [F, B]
    w16 = np.zeros((F, WP), dtype=np.float16)
    w16[:, :KD] = weights.astype(np.float16)
    combs = []
    for h in range(2):
        c = np.zeros((PC, 64), dtype=np.float16)
        for p in range(PC):
            c[p, p // D + 25 * h] = 1.0
        combs.append(c)
    ident = np.eye(128, dtype=np.float16)
    combi = np.zeros((128, 64), dtype=np.float16)
    for isub in range(2):
        for k in range(K):
            combi[isub * 64 + k, k] = 1.0
    in_maps = []
    for c in range(NCORES):
        blocks = [c, (c + 1) % 8, (c + 2) % 8, (c + 3) % 8, (c + 4) % 8]
        cols = np.concatenate([np.arange(b * IB, (b + 1) * IB)
                               for b in blocks])
        in_maps.append({
            "xt": np.ascontiguousarray(xt16[:, cols]),
            "w": w16,
            "bias": bias.astype(np.float32),
            "comb0": combs[0],
            "comb1": combs[1],
            "ident": ident,
            "combi": combi,
        })
    return in_maps


def kernel(x, weights, bias):
    x = np.ascontiguousarray(x, dtype=np.float32)
    weights = np.ascontiguousarray(weights, dtype=np.float32)
    bias = np.ascontiguousarray(bias, dtype=np.float32)

    if "nc" not in _CACHE:
        _CACHE["nc"] = _build()
    nc = _CACHE["nc"]

    in_maps = _in_maps(x, weights, bias)
    res = run_bass_kernel_spmd(nc, in_maps, list(range(NCORES)), trace=TRACE)
    _CACHE["last_res"] = res
    # unshard: place each core's row-features, then fold in the transposed
    # pairwise partials each core computed for its 3 ring-successor blocks.
    feats = np.concatenate([res.results[c]["feat"] for c in range(NCORES)],
                           axis=0).astype(np.float64)  # [B, K]
    for c in range(NCORES):
        colp = res.results[c]["colp"]  # [IB, 3, K]
        for d in (1, 2, 3):
            b = (c + d) % NCORES
            feats[b * IB:(b + 1) * IB] += colp[:, d - 1, :]
    return np.concatenate([x, feats.astype(np.float32)], axis=1)
